# revision 31
# baseline (speedup 1.0000x reference)
"""Trainium2 Bass kernel: multi-head attention with RoPE (causal), 8-core SPMD.

Sharding: 8 cores = 4 batches x 2 head-halves (tensor parallel over heads,
data parallel over batch). Each core computes, for its batch b and its 8
heads: QKV projections, RoPE, causal attention, and a partial output
projection. Host sums the two head-half partials per batch and adds the bias.

All TensorEngine math in fp16 with fp32 PSUM accumulation.

v2 changes vs the bf16 baseline:
- softmax denominator no longer uses a per-k-tile ones-matmul on the PE
  (which cost ~58us of PE stream); exp tiles are accumulated into a
  per-(h,qc) pt_sum on DVE/Pool and a single 512-col ones-matmul per
  (h,qc) produces the row sums.
- RoPE has no SBUF-SBUF swap DMAs: with cs=[cos;cos], sc=[sin;sin] the
  cross-partition combine is done directly by DVE tensor ops whose PSUM
  operand may use a different base partition than the SBUF operand
  (verifier only forbids misaligned bases when BOTH inputs are in SBUF).
- fp16 instead of bf16 (same PE rate, 2x DVE mode, more mantissa).
- head-0 weight loads are chunked so the first matmul starts ~2us in.
- V-projection PSUM drains moved to the Act engine (same act table as
  Exp, so no table-switch cost).

Schedule: attention(h) is interleaved with produce(h) at qc/block
granularity; the per-(h,qc) finalize (ones-matmul + reciprocal + norm)
is delayed a couple of items so the PE never waits on the DVE add chain.

DMA queues: weights + consts + xt qc1-3 + y write-back on sync; cs/sc and
half the y write-back on scalar; xt qc0 on gpsimd.
"""

import sys

for _p in ("/opt/trn_rl_repo",):
    if _p not in sys.path:
        sys.path.insert(0, _p)

import numpy as np

import concourse.bass as bass
import concourse.bacc as bacc
import concourse.mybir as mybir
import concourse.tile as tile
from concourse.bass_utils import run_bass_kernel_spmd

F16 = np.float16

B, T, C = 4, 2048, 2048
H = 16
D = C // H  # 128
ROPE_THETA = 1000000.0
N_CORES = 8
HPC = H // 2          # heads per core (8)
P = 128               # partitions
CHUNK = 512           # moving free dim per matmul
N_CC = C // P         # 16 contraction chunks
N_QC = T // CHUNK     # 4 q-chunks
N_KT = T // P         # 16 k-tiles
SCALE = 1.0 / float(np.sqrt(D))
NWC = 4               # weight-load chunks per projection matrix

_CACHED = {}


def build_kernel():
    """Build the SPMD Bass program (identical on all 8 cores)."""
    fp32 = mybir.dt.float32
    fp16 = mybir.dt.float16

    nc = bacc.Bacc("TRN2", target_bir_lowering=False, debug=False,
                   num_devices=N_CORES)

    # Per-core DRAM inputs (fp16 unless noted)
    xt = nc.dram_tensor("xt", [C, T], fp16, kind="ExternalInput")          # x[b].T
    wq = nc.dram_tensor("wq", [HPC, C, D], fp16, kind="ExternalInput")     # Wq_h.T (perm'd)
    wk = nc.dram_tensor("wk", [HPC, C, D], fp16, kind="ExternalInput")
    wv = nc.dram_tensor("wv", [HPC // 2, C, 2 * D], fp16, kind="ExternalInput")  # head pairs
    wo = nc.dram_tensor("wo", [HPC * D, C], fp16, kind="ExternalInput")    # WoT slice
    cs = nc.dram_tensor("cs", [P, T], fp16, kind="ExternalInput")          # [cosT;cosT]
    sc = nc.dram_tensor("sc", [P, T], fp16, kind="ExternalInput")          # [sinT;sinT]
    tri = nc.dram_tensor("tri", [P, P], fp16, kind="ExternalInput")        # k<=q mask
    y = nc.dram_tensor("y", [T, C], fp16, kind="ExternalOutput")

    Exp = mybir.ActivationFunctionType.Exp

    with tile.TileContext(nc) as tc:
        with (
            tc.tile_pool(name="const", bufs=1) as const_pool,
            tc.tile_pool(name="xtp", bufs=1) as xt_pool,
            tc.tile_pool(name="wqk", bufs=4) as wqk_pool,
            tc.tile_pool(name="wvp", bufs=2) as wv_pool,
            tc.tile_pool(name="bt", bufs=2) as b_pool,
            tc.tile_pool(name="qktr", bufs=2) as qk_pool,
            tc.tile_pool(name="vsb", bufs=2) as v_pool,
            tc.tile_pool(name="pt", bufs=4) as pt_pool,
            tc.tile_pool(name="pts", bufs=2) as pts_pool,
            tc.tile_pool(name="tmp", bufs=2) as tmp_pool,
            tc.tile_pool(name="rcp", bufs=1) as rcp_pool,
            tc.tile_pool(name="aot", bufs=1) as aot_pool,
            tc.tile_pool(name="wop", bufs=16) as wo_pool,
            tc.tile_pool(name="yst", bufs=3) as y_pool,
            tc.tile_pool(name="ph1", bufs=3, space="PSUM") as ph1_psum,
            tc.tile_pool(name="pst", bufs=2, space="PSUM") as s_psum,
            tc.tile_pool(name="po", bufs=1, space="PSUM") as o_psum,
        ):
            # ---- weight loads: chunked on the sync queue ----
            qk_w = {}

            def load_qk_w(h, mid=None):
                wq_r = wq.ap()[h].rearrange("(cc p) d -> p cc d", p=P)
                wk_r = wk.ap()[h].rearrange("(cc p) d -> p cc d", p=P)
                tiles = []
                for src in (wq_r, wk_r):
                    for ch in range(NWC):
                        t_sb = wqk_pool.tile([P, N_CC // NWC, D], fp16,
                                             tag=f"wqk{ch}")
                        nc.sync.dma_start(
                            out=t_sb,
                            in_=src[:, ch * (N_CC // NWC):(ch + 1) * (N_CC // NWC), :])
                        tiles.append(t_sb)
                    if src is wq_r and mid is not None:
                        mid()
                qk_w[h] = tiles  # [wq c0..c3, wk c0..c3]

            wv_w = {}

            def load_wv(pair):
                wv_r = wv.ap()[pair].rearrange("(cc p) d -> p cc d", p=P)
                tiles = []
                for ch in range(NWC):
                    t_sb = wv_pool.tile([P, N_CC // NWC, 2 * D], fp16,
                                        tag=f"wv{ch}")
                    nc.sync.dma_start(
                        out=t_sb,
                        in_=wv_r[:, ch * (N_CC // NWC):(ch + 1) * (N_CC // NWC), :])
                    tiles.append(t_sb)
                wv_w[pair] = tiles

            # ---- x^T loader. No DMAs ever go on the gpsimd queue: an
            # unused sw-DGE makes the end-of-program drain cheap. ----
            xt_t = {}
            xt_r = xt.ap().rearrange("(cc p) t -> p cc t", p=P)

            def load_xt(cc, qc, eng):
                t_sb = xt_pool.tile([P, CHUNK], fp16, tag=f"xt{cc}_{qc}",
                                    name=f"xt{cc}_{qc}")
                eng.dma_start(out=t_sb,
                              in_=xt_r[:, cc, qc * CHUNK:(qc + 1) * CHUNK])
                xt_t[(cc, qc)] = t_sb

            # head-0 weights with early xt tiles interleaved (sync queue);
            # xt cc0/cc1 + RoPE consts on the scalar queue in parallel
            load_qk_w(0, mid=lambda: [load_xt(cc, 0, nc.sync)
                                      for cc in range(2, 8)])
            cs_sb = const_pool.tile([P, T], fp16)
            sc_sb = const_pool.tile([P, T], fp16)
            tri2_sb = const_pool.tile([P, 2, P], fp16)
            ones_sb = const_pool.tile([P, P], fp16)
            load_xt(0, 0, nc.scalar)
            load_xt(1, 0, nc.scalar)
            nc.scalar.dma_start(out=cs_sb, in_=cs.ap())
            nc.scalar.dma_start(out=sc_sb, in_=sc.ap())
            nc.scalar.dma_start(out=tri2_sb[:, 0, :], in_=tri.ap())
            nc.scalar.dma_start(out=tri2_sb[:, 1, :], in_=tri.ap())
            nc.vector.memset(ones_sb, 1.0)
            for cc in range(8, N_CC):
                load_xt(cc, 0, nc.sync)
            load_wv(0)
            for qc in range(1, N_QC):
                for cc in range(N_CC):
                    load_xt(cc, qc, nc.sync)

            aot_sb = aot_pool.tile([P, HPC, T], fp16)  # attn-out^T, all heads

            # ---- work items ----
            def qk_block(w_chunks, out_sb, qc):
                """One 512-wide projection block with RoPE applied.

                RoPE entirely on DVE, with no partition-swap copies: with
                cs=[cos;cos] and the sign-folded sc=[-sin;sin],
                  ra        = ps * cs               (fused drain+mul STT)
                  rb[0:64]  = ps[64:128] * sc[0:64]   = -o*sin
                  rb[64:128]= ps[0:64]  * sc[64:128]  =  e*sin
                  out = ra + rb -> [re; ro]
                The half-multiplies read the PSUM block at a different base
                partition than their SBUF operand, which the ISA allows
                (only SBUF+SBUF input pairs must have equal bases). Keeping
                the whole chain on one engine removes cross-engine queue
                latency from the K->scores critical path.
                """
                ps = ph1_psum.tile([P, CHUNK], fp32, tag="ph1")
                for cc in range(N_CC):
                    nc.tensor.matmul(
                        ps, lhsT=w_chunks[cc // NWC][:, cc % NWC, :],
                        rhs=xt_t[(cc, qc)],
                        start=(cc == 0), stop=(cc == N_CC - 1))
                sl = slice(qc * CHUNK, (qc + 1) * CHUNK)
                ra = b_pool.tile([P, CHUNK], fp16, tag="ra")
                rb = b_pool.tile([P, CHUNK], fp16, tag="rb")
                # fused drain+multiply: ra = ps * cs in one DVE op
                nc.vector.scalar_tensor_tensor(
                    ra, ps, 1.0, cs_sb[:, sl],
                    mybir.AluOpType.bypass, mybir.AluOpType.mult)
                nc.vector.tensor_mul(rb[0:64, :], ps[64:128, :],
                                     sc_sb[0:64, sl])
                nc.vector.tensor_mul(rb[64:128, :], ps[0:64, :],
                                     sc_sb[64:128, sl])
                # sc = [-sin; sin], so one full-width add yields [re; ro]
                nc.vector.tensor_add(out_sb[:, sl], ra, rb)

            def v_block(w_chunks, v_sb, tt0):
                """Four t-tiles of the V projection for one head pair."""
                for tt in range(tt0, tt0 + 4):
                    ps = ph1_psum.tile([P, CHUNK], fp32, tag="ph1")
                    psv = ps[:, 0:2 * D]
                    t0 = (tt % 4) * P
                    for cc in range(N_CC):
                        nc.tensor.matmul(
                            psv, lhsT=xt_t[(cc, tt // 4)][:, t0:t0 + P],
                            rhs=w_chunks[cc // NWC][:, cc % NWC, :],
                            start=(cc == 0), stop=(cc == N_CC - 1))
                    # alternate drains Act/DVE so a burst on either engine
                    # doesn't hold the ph1 ring
                    if tt % 2 == 0:
                        nc.vector.tensor_scalar_add(v_sb[:, tt, :], psv, 0.0)
                    else:
                        nc.scalar.copy(v_sb[:, tt, :], psv)

            v_tiles = {}
            qk_tiles = {}

            def produce_items(h):
                w_tiles = qk_w.pop(h)
                qtr = qk_pool.tile([P, T], fp16, tag="wqtr", name=f"qtr{h}")
                ktr = qk_pool.tile([P, T], fp16, tag="wktr", name=f"ktr{h}")
                qk_tiles[h] = (qtr, ktr)
                items = []
                for qc in range(N_QC):
                    items.append(lambda qc=qc: qk_block(w_tiles[:NWC], qtr, qc))
                for qc in range(N_QC):
                    items.append(lambda qc=qc: qk_block(w_tiles[NWC:], ktr, qc))
                if h % 2 == 0:
                    w_v = wv_w.pop(h // 2)
                    v_sb = v_pool.tile([P, N_KT, 2 * D], fp16, tag="vsb",
                                       name=f"vsb{h // 2}")
                    v_tiles[h // 2] = v_sb
                    for g in range(4):
                        items.append(lambda g=g, w=w_v, v=v_sb: v_block(w, v, 4 * g))
                return items

            fin_info = {}

            att_state = {}

            def attention_p1(h, qc, qtr, ktr):
                """Diagonal scores + exps + masks + denominator accs.

                The four trimmed diagonal tiles are computed as two PSUM
                pairs with one exp each (trimmed slots hold stale-PSUM
                garbage that is never read). Their PV matmuls run at the
                END of part2, so the exp/mask chain is hidden behind a
                whole produce item of PE work.
                """
                q0 = qc * CHUNK
                pt_sum = pts_pool.tile([P, CHUNK], fp16, tag="pts")
                diag = []
                for dp in range(2):
                    ps2 = s_psum.tile([P, 2, CHUNK], fp32, tag="pst")
                    for i in range(2):
                        j = 2 * dp + i
                        n0 = j * P
                        nc.tensor.matmul(
                            ps2[:, i, 0:CHUNK - n0],
                            lhsT=ktr[:, (4 * qc + j) * P:(4 * qc + j + 1) * P],
                            rhs=qtr[:, q0 + n0:q0 + CHUNK],
                            start=True, stop=True)
                    pt2 = pt_pool.tile([P, 2, CHUNK], fp16, tag="pt")
                    nc.scalar.activation(pt2, ps2, Exp, scale=SCALE)
                    # both slots' triangle masks in one DVE op
                    nc.vector.tensor_mul(pt2[:, :, 0:P], pt2[:, :, 0:P],
                                         tri2_sb)
                    diag.append(pt2)
                nc.vector.tensor_scalar_add(pt_sum, diag[0][:, 0, :], 0.0)
                for j in (1, 2, 3):
                    n0 = j * P
                    nc.vector.tensor_add(
                        pt_sum[:, n0:], pt_sum[:, n0:],
                        diag[j // 2][:, j % 2, 0:CHUNK - n0])
                att_state[(h, qc)] = (pt_sum, diag)

            def attention_p2(h, qc, qtr, ktr, v_sb, v_col):
                """Off-diagonal pair pipeline + all PV matmuls."""
                q0 = qc * CHUNK
                pt_sum, diag = att_state.pop((h, qc))
                ps_o = o_psum.tile([P, CHUNK], fp32, tag="po")
                n_kt = 4 * qc + 4

                def v_mm(kt, pt_ap, n0):
                    nc.tensor.matmul(
                        ps_o[:, n0:CHUNK],
                        lhsT=v_sb[:, kt, v_col * D:(v_col + 1) * D],
                        rhs=pt_ap, start=(kt == 0), stop=(kt == n_kt - 1))

                for kt in range(0, 4 * qc, 2):
                    ps2 = s_psum.tile([P, 2, CHUNK], fp32, tag="pst")
                    for i in range(2):
                        nc.tensor.matmul(
                            ps2[:, i, :],
                            lhsT=ktr[:, (kt + i) * P:(kt + i + 1) * P],
                            rhs=qtr[:, q0:q0 + CHUNK],
                            start=True, stop=True)
                    pt2 = pt_pool.tile([P, 2, CHUNK], fp16, tag="pt")
                    nc.scalar.activation(pt2, ps2, Exp, scale=SCALE)
                    if qc == 3:
                        # qc3 feeds the finalize critical path: two direct
                        # DVE adds beat the higher-latency Pool assist
                        nc.vector.tensor_add(pt_sum, pt_sum, pt2[:, 0, :])
                        nc.vector.tensor_add(pt_sum, pt_sum, pt2[:, 1, :])
                    else:
                        # pair-internal sum on Pool, += on DVE
                        tmp = tmp_pool.tile([P, CHUNK], fp16, tag="tmp")
                        nc.gpsimd.tensor_add(tmp, pt2[:, 0, :], pt2[:, 1, :])
                        nc.vector.tensor_add(pt_sum, pt_sum, tmp)
                    for i in range(2):
                        v_mm(kt + i, pt2[:, i, :], 0)
                # diagonal PV matmuls last; their exps are long done
                for j in range(4):
                    n0 = j * P
                    v_mm(4 * qc + j, diag[j // 2][:, j % 2, 0:CHUNK - n0], n0)
                fin_info[(h, qc)] = (ps_o, pt_sum)

            def finalize_qc(h, qc):
                """Denominator ones-matmul + reciprocal + normalization.

                The ones-matmul result time-shares the scores PSUM pool
                (its bank budget went to a third ph1 buffer instead)."""
                ps_o, pt_sum = fin_info.pop((h, qc))
                q0 = qc * CHUNK
                ps_rt = s_psum.tile([P, 2, CHUNK], fp32, tag="pst",
                                    name=f"psr{h}_{qc}")
                ps_r = ps_rt[:, 0, :]
                nc.tensor.matmul(ps_r, lhsT=ones_sb, rhs=pt_sum,
                                 start=True, stop=True)
                rec = rcp_pool.tile([P, CHUNK], fp32, tag="rcp")
                nc.vector.reciprocal_approx_fast(out=rec, in_=ps_r)
                nc.vector.tensor_mul(aot_sb[:, h, q0:q0 + CHUNK], ps_o, rec)

            def attention_items(h):
                qtr, ktr = qk_tiles.pop(h)
                v_sb = v_tiles[h // 2]
                v_col = h % 2
                a1 = [lambda qc=qc: attention_p1(h, qc, qtr, ktr)
                      for qc in range(N_QC)]
                a2 = [lambda qc=qc: attention_p2(h, qc, qtr, ktr, v_sb, v_col)
                      for qc in range(N_QC)]
                return a1, a2

            wo_tiles = {}

            def load_wo(co):
                tiles = []
                for cb in range(HPC):  # contraction chunks == heads
                    w_sb = wo_pool.tile([P, CHUNK], fp16, tag="wo")
                    nc.sync.dma_start(
                        out=w_sb,
                        in_=wo.ap()[cb * P:(cb + 1) * P,
                                    co * CHUNK:(co + 1) * CHUNK])
                    tiles.append(w_sb)
                wo_tiles[co] = tiles

            def ph3_group(co, g, last_group=False):
                """Four t-tiles of the output projection for column group co."""
                tiles = wo_tiles[co]
                for tt in range(4 * g, 4 * g + 4):
                    ps = ph1_psum.tile([P, CHUNK], fp32, tag="ph1")
                    for cb in range(HPC):
                        nc.tensor.matmul(
                            ps, lhsT=aot_sb[:, cb, tt * P:(tt + 1) * P],
                            rhs=tiles[cb],
                            start=(cb == 0), stop=(cb == HPC - 1))
                    yt = y_pool.tile([P, CHUNK], fp16, tag="y")
                    # alternate drains between Act and DVE so neither
                    # engine's in-order stream delays interleaved attention
                    if tt % 2 == 0:
                        nc.vector.tensor_scalar_add(yt, ps, 0.0)
                    else:
                        nc.scalar.copy(yt, ps)
                    if last_group:
                        # final write-backs alternate the two hw queues
                        deng = (nc.sync, nc.scalar, nc.sync, nc.scalar)[tt % 4]
                    else:
                        deng = nc.sync if tt % 2 == 0 else nc.scalar
                    deng.dma_start(
                        out=y.ap()[tt * P:(tt + 1) * P,
                                   co * CHUNK:(co + 1) * CHUNK],
                        in_=yt)

            # ---- head steps: produce(h) gate-interleaved with attention(h).
            # attention(h).qc needs only Q-block qc, K-blocks <= qc and
            # V-groups <= qc of its own head, so it lags produce(h) by a
            # couple of items. finalize(h,qc) lags attention(h,qc) by a
            # produce item so the PE's ones-matmul never waits on the DVE
            # accumulation chain.
            pend = []  # delayed finalize carried into the next head

            def flush_pend():
                while pend:
                    pend.pop(0)()

            for h in range(HPC):
                if h + 1 < HPC:
                    load_qk_w(h + 1)
                    if (h + 1) % 2 == 0:
                        load_wv((h + 1) // 2)
                if h == HPC - 2:
                    load_wo(0)
                if h == HPC - 1:
                    load_wo(1)
                p = produce_items(h)     # [Q0..Q3, K0..K3, (V0..V3)]
                a1, a2 = attention_items(h)
                f = [lambda qc=qc, h=h: finalize_qc(h, qc)
                     for qc in range(N_QC)]
                last = h == HPC - 1
                # every consumer has at least one substantial produce item
                # between it and its producer: K_qc -> (spacer) -> a1_qc,
                # a1_qc -> (spacer) -> a2_qc, a2_qc -> (spacer) -> f_qc
                if len(p) == 12:  # even head: Q, K, V blocks
                    seq = [p[0], flush_pend, p[4], p[8], a1[0], p[1],
                           a2[0], p[5], f[0], p[9], a1[1], p[2],
                           a2[1], p[6], f[1], p[10], a1[2], p[3],
                           a2[2], p[7], f[2], p[11], a1[3], a2[3]]
                    for s in seq:
                        s()
                    pend.append(f[3])
                elif not last:    # odd head: Q, K only
                    seq = [p[0], flush_pend, p[4], p[1], a1[0], p[5],
                           a2[0], p[2], f[0], a1[1], p[6],
                           a2[1], p[3], f[1], a1[2], p[7],
                           a2[2], a1[3], f[2], a2[3]]
                    for s in seq:
                        s()
                    pend.append(f[3])
                else:
                    # head 7: phase-3 groups give the Act/DVE chains slack
                    # to drain the attention tail
                    seq = [p[0], flush_pend, p[4], p[1], a1[0], p[5],
                           a2[0], p[2], f[0], a1[1], p[6],
                           a2[1], p[3], f[1], a1[2], p[7],
                           a2[2], a1[3], f[2], a2[3]]
                    for s in seq:
                        s()
                    ph3_group(0, 0)
                    ph3_group(0, 1)
                    f[3]()
                    for g in range(2, 4):
                        ph3_group(0, g)

            # ---- phase 3: remaining output-projection column groups ----
            for co in range(1, N_QC):
                if co + 1 < N_QC:
                    load_wo(co + 1)
                for g in range(4):
                    ph3_group(co, g, last_group=(co == N_QC - 1 and g == 3))

    nc.finalize()
    return nc


def _host_prep(x, Wq, Wk, Wv, Wo):
    """Build the 8 per-core input maps."""
    perm = np.concatenate([np.arange(0, D, 2), np.arange(1, D, 2)])

    inv_freq = 1.0 / ROPE_THETA ** (np.arange(0, D, 2, dtype=np.float32) / D)
    pos = np.arange(T, dtype=np.float32)
    freqs = np.einsum("i,j->ij", pos, inv_freq)  # [T, 64]
    cosT = np.cos(freqs).T.astype(np.float32)
    sinT = np.sin(freqs).T.astype(np.float32)
    cs = np.concatenate([cosT, cosT], 0).astype(F16)
    sc = np.concatenate([-sinT, sinT], 0).astype(F16)
    tri = (np.arange(P)[:, None] <= np.arange(P)[None, :]).astype(F16)

    halves = []
    for g in range(2):
        heads = range(g * HPC, (g + 1) * HPC)
        wq_g = np.stack([Wq[h * D:(h + 1) * D, :][perm, :].T.astype(F16)
                         for h in heads])                       # [8, C, D]
        wk_g = np.stack([Wk[h * D:(h + 1) * D, :][perm, :].T.astype(F16)
                         for h in heads])
        wv_g = np.stack([Wv[(g * HPC + 2 * p) * D:(g * HPC + 2 * p + 2) * D, :]
                         .T.astype(F16) for p in range(HPC // 2)])  # [4, C, 2D]
        wo_g = Wo[:, g * HPC * D:(g + 1) * HPC * D].T.astype(F16)   # [1024, C]
        halves.append(dict(wq=wq_g, wk=wk_g, wv=wv_g, wo=wo_g))

    in_maps = []
    for core in range(N_CORES):
        b, g = core // 2, core % 2
        m = dict(halves[g])
        m["xt"] = np.ascontiguousarray(x[b].T).astype(F16)
        m["cs"] = cs
        m["sc"] = sc
        m["tri"] = tri
        in_maps.append(m)
    return in_maps


def kernel(x, Wq, Wk, Wv, Wo, bo):
    x = np.asarray(x, dtype=np.float32)
    Wq = np.asarray(Wq, dtype=np.float32)
    Wk = np.asarray(Wk, dtype=np.float32)
    Wv = np.asarray(Wv, dtype=np.float32)
    Wo = np.asarray(Wo, dtype=np.float32)
    bo = np.asarray(bo, dtype=np.float32)

    if "nc" not in _CACHED:
        _CACHED["nc"] = build_kernel()
    nc = _CACHED["nc"]

    in_maps = _host_prep(x, Wq, Wk, Wv, Wo)
    res = run_bass_kernel_spmd(nc, in_maps, core_ids=list(range(N_CORES)))

    out = np.empty((B, T, C), np.float32)
    for b in range(B):
        out[b] = (res.results[2 * b]["y"].astype(np.float32)
                  + res.results[2 * b + 1]["y"].astype(np.float32) + bo)
    return out
